# revision 10
# baseline (speedup 1.0000x reference)
"""CompoundProteinInteractionPrediction on 8 Trainium2 NeuronCores (Bass/Tile).

v3 restructure (vs v2):
- Inputs packed into 3 tensors (bmat / packbf / packf32) — per-arg dispatch
  cost through the axon client is significant.
- compound = mean(xs3) computed via host-side global colsum of A:
  mean(xs3) = mean(xs0) + (cs.hs0 + cs.hs1 + cs.hs2)/N. dot0/dot1 are
  computed locally on every core (hs0 from replicated xs0; hs1 post
  AllGather), accumulated on PE during the SpMM loops. Only dot2 needs a
  cross-core reduction.
- The protein attention features hs_p (independent of the GNN) and the
  dot2 partial (hi/lo bf16 split for ~f32 accuracy) ride ONE AllGather;
  each core then computes the exact tanh attention + output MLP locally.
  This removes v2's ReduceScatter and both tail AllReduces — per rep only
  two collectives remain (AG hs1, AG hs_p|dot2), both substantially
  overlapped with SpMM/conv work.
- reps>1 runs the whole model body repeatedly inside one NEFF for
  steady-state throughput timing (amortizes per-execution launch costs).

Engine assignment: slab DMAs on SP/ACT queues (alternating), collective
input + readback DMAs on the gpsimd queue so slab prefetch is never queued
behind a collective-blocked dma_start.
"""
import sys

sys.path.insert(0, "/opt/trn_rl_repo")

import numpy as np
import ml_dtypes

import concourse.bass as bass
import concourse.tile as tile
from concourse import bacc, mybir

F8 = ml_dtypes.float8_e4m3
BF16 = ml_dtypes.bfloat16

DIM = 128
WINDOW = 5
KK = 2 * WINDOW + 1
LAYER_GNN = 3
LAYER_CNN = 3
LAYER_OUT = 2
HALO = WINDOW * LAYER_CNN  # 15
HS_SCALES = (1024.0, 128.0, 16.0)  # per-layer fp8 scale for hs (max ~140-165)

FULL = dict(na=16384, nw=16384, nfp=100000, nword=20000, ncores=8)


def _ceil_div(a, b):
    return (a + b - 1) // b


def build_kernel(na, nw, nfp, nword, ncores, reps=1, stage=7,
                 dma_mode="alt", slab_bufs=5, cache_pairs=5):
    local_a = na // ncores
    local_w = nw // ncores
    ach = na // 128            # 128 atom chunks of 128
    och = local_a // 128       # 16 own chunks per core
    lwin = local_w + 2 * HALO
    wch = _ceil_div(lwin, 128)
    wpad = wch * 128
    n_mb = local_a // 512      # 4 psum column tiles of 512

    f32 = mybir.dt.float32
    bf16 = mybir.dt.bfloat16
    f8 = mybir.dt.float8e4
    Relu = mybir.ActivationFunctionType.Relu
    Tanh = mybir.ActivationFunctionType.Tanh
    Ident = mybir.ActivationFunctionType.Identity
    DR = mybir.MatmulPerfMode.DoubleRow
    Add = mybir.AluOpType.add
    Sub = mybir.AluOpType.subtract
    Mult = mybir.AluOpType.mult
    AxX = mybir.AxisListType.X

    nc = bacc.Bacc("TRN2", target_bir_lowering=False, debug=False,
                   enable_asserts=False, num_devices=ncores)

    # ---- DRAM inputs (per-core values via in_maps) ----
    # packf32 columns: [wgT 0:128 | woT blocks 128:640 | bo 640:642 |
    #   wiT 642:646 | ba 646 | convb 647 | bi(rows 0:2) 648 | bg(row 0)
    #   649:777 | csg_all 777:905 | csg_own 905:921 | mean_xs0 921]
    # packbf columns: [waT 0:128 | convm 128:1536 | xs0T 1536:1536+na |
    #   xs0ownT +local_a | img0 +wpad | wmask +wpad]
    PF = 922
    PBF = 1536 + na + local_a + 2 * wpad
    ob_xs0own = 1536 + na
    ob_img0 = ob_xs0own + local_a
    ob_wmask = ob_img0 + wpad
    t_bmat = nc.dram_tensor("bmat", [na, local_a], f8, kind="ExternalInput").ap()
    t_packbf = nc.dram_tensor("packbf", [128, PBF], bf16, kind="ExternalInput").ap()
    t_packf32 = nc.dram_tensor("packf32", [128, PF], f32, kind="ExternalInput").ap()
    t_out = nc.dram_tensor("out", [2, 1], f32, kind="ExternalOutput").ap()

    rg = [list(range(ncores))]
    bmat_r = t_bmat.rearrange("(t p) m -> t p m", p=128)

    with tile.TileContext(nc) as tc:
        with (
            tc.tile_pool(name="persist", bufs=1) as persist,
            tc.tile_pool(name="hsp", bufs=3) as hsp,
            tc.tile_pool(name="slabp", bufs=slab_bufs) as slabp,
            tc.tile_pool(name="xsp", bufs=2) as xsp,
            tc.tile_pool(name="convp", bufs=3) as convp,
            tc.tile_pool(name="hsfp", bufs=1) as hsfp,
            tc.tile_pool(name="agbp", bufs=2) as agbp,
            tc.tile_pool(name="stp", bufs=3) as stp,
            tc.tile_pool(name="smallp", bufs=12) as smallp,
            tc.tile_pool(name="dotp", bufs=3) as dotp,
            tc.tile_pool(name="catp", bufs=3) as catp,
            tc.tile_pool(name="dram", bufs=1, space="DRAM") as dram,
            tc.tile_pool(name="ps_spmm", bufs=1, space="PSUM") as ps_spmm,
            tc.tile_pool(name="ps_misc", bufs=2, space="PSUM") as ps_misc,
            tc.tile_pool(name="ps_conv", bufs=1, space="PSUM") as ps_conv,
            tc.tile_pool(name="ps_dot", bufs=1, space="PSUM") as ps_dot,
        ):
            # ---- persistent small weights ----
            wgT = persist.tile([DIM, DIM], f32, tag="wgT")
            wgT_bf = persist.tile([DIM, DIM], bf16, tag="wgT_bf")
            bg_row = persist.tile([1, DIM], f32, tag="bg_row")
            waT = persist.tile([DIM, DIM], bf16, tag="waT")
            ba_col = persist.tile([DIM, 1], f32, tag="ba_col")
            convm = persist.tile([DIM, KK * DIM], bf16, tag="convm")
            convb_col = persist.tile([DIM, 1], f32, tag="convb_col")
            woT_sb = persist.tile([DIM, 4 * DIM], f32, tag="woT_sb")
            bo_sb = persist.tile([DIM, 2], f32, tag="bo_sb")
            wiT_sb = persist.tile([DIM, 4], f32, tag="wiT_sb")
            bi_sb = persist.tile([2, 1], f32, tag="bi_sb")
            ones_row = persist.tile([1, DIM], bf16, tag="ones_row")
            ones_f32 = persist.tile([1, DIM], f32, tag="ones_f32")
            bg_bf = persist.tile([1, DIM], bf16, tag="bg_bf")
            xs0_full = persist.tile([128, na], bf16, tag="xs0_full")
            xs0_own = persist.tile([128, local_a], bf16, tag="xs0_own")
            img0 = persist.tile([128, wpad], bf16, tag="img0")
            wmask_sb = persist.tile([128, wpad], bf16, tag="wmask_sb")
            csga_f32 = persist.tile([DIM, DIM], f32, tag="csga_f32")
            csga_bf = persist.tile([DIM, DIM], bf16, tag="csga_bf")
            csgo_f32 = persist.tile([DIM, och], f32, tag="csgo_f32")
            csgo_bf = persist.tile([DIM, och], bf16, tag="csgo_bf")
            mean_xs0 = persist.tile([DIM, 1], f32, tag="mean_xs0")
            bcache = (persist.tile([128, cache_pairs * 2 * local_a], f8,
                                   tag="bcache", name="bcache") if cache_pairs else None)

            nc.sync.dma_start(wgT[:], t_packf32[:, 0:128])
            nc.vector.tensor_copy(wgT_bf[:], wgT[:])
            nc.sync.dma_start(bg_row[:], t_packf32[0:1, 649:777])
            nc.sync.dma_start(waT[:], t_packbf[:, 0:128])
            nc.sync.dma_start(ba_col[:], t_packf32[:, 646:647])
            nc.sync.dma_start(convm[:], t_packbf[:, 128:1536])
            nc.sync.dma_start(convb_col[:], t_packf32[:, 647:648])
            nc.sync.dma_start(woT_sb[:], t_packf32[:, 128:640])
            nc.sync.dma_start(bo_sb[:], t_packf32[:, 640:642])
            nc.sync.dma_start(wiT_sb[:], t_packf32[:, 642:646])
            nc.sync.dma_start(bi_sb[:], t_packf32[0:2, 648:649])
            nc.sync.dma_start(csga_f32[:], t_packf32[:, 777:905])
            nc.vector.tensor_copy(csga_bf[:], csga_f32[:])
            nc.sync.dma_start(csgo_f32[:], t_packf32[:, 905:921])
            nc.vector.tensor_copy(csgo_bf[:], csgo_f32[:])
            nc.sync.dma_start(mean_xs0[:], t_packf32[:, 921:922])
            nc.gpsimd.memset(ones_f32[:], 1.0)
            nc.vector.tensor_copy(ones_row[:], ones_f32[:])
            nc.vector.tensor_copy(bg_bf[:], bg_row[:])
            nc.sync.dma_start(xs0_full[:], t_packbf[:, 1536:1536 + na])
            nc.sync.dma_start(xs0_own[:],
                              t_packbf[:, ob_xs0own:ob_xs0own + local_a])
            nc.sync.dma_start(img0[:], t_packbf[:, ob_img0:ob_img0 + wpad])
            nc.sync.dma_start(wmask_sb[:], t_packbf[:, ob_wmask:ob_wmask + wpad])

            def spmm_layer(layer, rep, hs_src, extra=None):
                """SpMM over own rows via fp8 DoubleRow; psum [dims, local_a].
                hs_src(kj) -> lhsT AP [128, 2, DIM] f8 for chunk pair kj.
                extra(kj), if given, is emitted once per pair (dot matmuls)."""
                psums = [ps_spmm.tile([128, 512], f32, tag=f"spmm{mb}",
                                      name=f"spmm_r{rep}_l{layer}_{mb}")
                         for mb in range(n_mb)]
                kc0 = ach // 2 - cache_pairs
                for kj in range(ach // 2):
                    if cache_pairs and kj >= kc0:
                        slab = bcache[:, (kj - kc0) * 2 * local_a:
                                      (kj - kc0 + 1) * 2 * local_a]
                        if layer == 0:
                            eng = nc.sync if kj % 2 == 0 else nc.scalar
                            eng.dma_start(slab[:, :local_a], bmat_r[2 * kj])
                            eng.dma_start(slab[:, local_a:], bmat_r[2 * kj + 1])
                        slab3 = slab.rearrange("p (two m) -> p two m", two=2)
                    else:
                        slab = slabp.tile([128, 2 * local_a], f8, tag="slab")
                        if dma_mode == "sp":
                            nc.sync.dma_start(slab[:, :local_a], bmat_r[2 * kj])
                            nc.sync.dma_start(slab[:, local_a:], bmat_r[2 * kj + 1])
                        elif dma_mode == "alt":
                            eng = nc.sync if kj % 2 == 0 else nc.scalar
                            eng.dma_start(slab[:, :local_a], bmat_r[2 * kj])
                            eng.dma_start(slab[:, local_a:], bmat_r[2 * kj + 1])
                        else:  # "half": one half per engine every pair
                            nc.sync.dma_start(slab[:, :local_a], bmat_r[2 * kj])
                            nc.scalar.dma_start(slab[:, local_a:], bmat_r[2 * kj + 1])
                        slab3 = slab[:].rearrange("p (two m) -> p two m", two=2)
                    lhsT = hs_src(kj)
                    for mb in range(n_mb):
                        nc.tensor.matmul(
                            psums[mb][:], lhsT, slab3[:, :, mb * 512:(mb + 1) * 512],
                            start=(kj == 0), stop=(kj == ach // 2 - 1),
                            perf_mode=DR)
                    if extra is not None:
                        extra(kj)
                return psums

            def own_hs(xs_c, rep, layer):
                """hs (fp8, scaled) for own atoms from xs_c [128(dim), local_a]."""
                hso = hsp.tile([128, och * 128], f8, tag="hso",
                               name=f"hso_r{rep}_l{layer}")
                for t in range(och):
                    hp = ps_misc.tile([128, 256], f32, tag="m256")
                    nc.tensor.matmul(hp[:, :128], ones_f32[:], bg_row[:],
                                     start=True, stop=False)
                    nc.tensor.matmul(hp[:, :128], xs_c[:, t * 128:(t + 1) * 128],
                                     wgT[:], start=False, stop=True)
                    nc.scalar.activation(hso[:, t * 128:(t + 1) * 128], hp[:, :128],
                                         Relu, scale=HS_SCALES[layer])
                return hso

            def allgather_hs(hso, rep, layer):
                """own hs f8 [128, 2048] -> hs_full f8 [128, ach*128].
                All DMAs on gpsimd queue (keeps SP/ACT slab prefetch clear)."""
                agi = dram.tile([128, och * 128], f8, tag=f"agi_r{rep}_l{layer}",
                                name=f"agi_r{rep}_l{layer}")
                ago = dram.tile([128 * ncores, och * 128], f8,
                                tag=f"ago_r{rep}_l{layer}",
                                name=f"ago_r{rep}_l{layer}", addr_space="Shared")
                nc.gpsimd.dma_start(agi[:], hso[:])
                nc.gpsimd.collective_compute(
                    "AllGather", mybir.AluOpType.bypass,
                    ins=[agi[:].opt()], outs=[ago[:].opt()], replica_groups=rg)
                hs_full = hsfp.tile([128, ach * 128], f8, tag="hsfull",
                                    name=f"hsfull_r{rep}_l{layer}")
                for c in range(ncores):
                    nc.gpsimd.dma_start(
                        hs_full[:, c * och * 128:(c + 1) * och * 128],
                        ago[c * 128:(c + 1) * 128, :])
                return hs_full

            def xs_update(psums, base_bf, rep, layer):
                """xs_new f32 = base + psum/HS_SCALES[layer]."""
                xs_c = xsp.tile([128, local_a], f32, tag="xs_c",
                                name=f"xs_r{rep}_l{layer}")
                for mb in range(n_mb):
                    nc.vector.scalar_tensor_tensor(
                        xs_c[:, mb * 512:(mb + 1) * 512], psums[mb][:],
                        1.0 / HS_SCALES[layer], base_bf[:, mb * 512:(mb + 1) * 512],
                        op0=Mult, op1=Add)
                return xs_c

            # ---------------- protein conv pieces ----------------
            def conv_layer(src, l, rep):
                lo = WINDOW * (l + 1)
                hi = lwin - WINDOW * (l + 1)
                dst = convp.tile([128, wpad], bf16, tag="convb",
                                 name=f"conv_r{rep}_l{l}")
                o = lo
                while o < hi:
                    ms = min(512, hi - o)
                    pc = ps_conv.tile([128, 512], f32, tag="pconv")
                    for a in range(KK):
                        nc.tensor.matmul(
                            pc[:, :ms], convm[:, a * DIM:(a + 1) * DIM],
                            src[:, o + a - WINDOW:o + a - WINDOW + ms],
                            start=(a == 0), stop=(a == KK - 1))
                    nc.scalar.activation(dst[:, o:o + ms], pc[:, :ms], Relu,
                                         bias=convb_col[:])
                    o += ms
                if l < LAYER_CNN - 1:
                    nc.vector.tensor_tensor(dst[:, lo:hi], dst[:, lo:hi],
                                            wmask_sb[:, lo:hi], op=Mult)
                return dst

            def protein_hsp(xsp_img, dst):
                """hs_p = relu(Wa xs_p + ba) for own words -> dst[:, :local_w]."""
                o = 0
                while o < local_w:
                    ms = min(512, local_w - o)
                    pc = ps_conv.tile([128, 512], f32, tag="pconv")
                    nc.tensor.matmul(pc[:, :ms], waT[:],
                                     xsp_img[:, HALO + o:HALO + o + ms],
                                     start=True, stop=True)
                    nc.scalar.activation(dst[:, o:o + ms], pc[:, :ms], Relu,
                                         bias=ba_col[:])
                    o += ms

            def tail(rep, dot0_sb, dot1_sb, dots16, hs_p_full):
                # dot2 = sum of gathered hi/lo partials (16 bf16 cols)
                d2s = smallp.tile([128, 1], f32, tag="small")
                nc.vector.reduce_sum(d2s[:], dots16[:], axis=AxX)
                # transpose dot0+dot1 rows -> [128,1] column, accumulated
                pdt = ps_misc.tile([128, 256], f32, tag="m256")
                nc.tensor.matmul(pdt[:, :1], dot0_sb[:], ones_f32[0:1, 0:1],
                                 start=True, stop=False)
                nc.tensor.matmul(pdt[:, :1], dot1_sb[:], ones_f32[0:1, 0:1],
                                 start=False, stop=True)
                t1 = smallp.tile([128, 1], f32, tag="small")
                nc.vector.tensor_tensor(t1[:], pdt[:, :1], d2s[:], op=Add)
                comp = smallp.tile([128, 1], f32, tag="small")
                nc.vector.scalar_tensor_tensor(comp[:], t1[:], 1.0 / na,
                                               mean_xs0[:], op0=Mult, op1=Add)
                # h = relu(Wa compound + ba)
                comp_bf = smallp.tile([128, 1], bf16, tag="smallbf")
                nc.vector.tensor_copy(comp_bf[:], comp[:])
                ph = ps_conv.tile([128, 512], f32, tag="pconv")
                nc.tensor.matmul(ph[:, :1], waT[:], comp_bf[:], start=True,
                                 stop=True)
                h_bf = smallp.tile([128, 1], bf16, tag="smallbf")
                nc.scalar.activation(h_bf[:], ph[:, :1], Relu, bias=ba_col[:])
                # exact attention over ALL nw words (hs_p_full gathered)
                prev = None
                o = 0
                while o < nw:
                    ms = min(256, nw - o)
                    pw = ps_misc.tile([128, 256], f32, tag="m256")
                    nc.tensor.matmul(pw[:1, :ms], h_bf[:],
                                     hs_p_full[:, o:o + ms], start=True, stop=True)
                    w_bf = stp.tile([1, 256], bf16, tag="wrow")
                    nc.scalar.activation(w_bf[:, :ms], pw[:1, :ms], Tanh)
                    pb = ps_misc.tile([128, 256], f32, tag="m256")
                    nc.tensor.matmul(pb[:, :ms], ones_row[:], w_bf[:, :ms],
                                     start=True, stop=True)
                    scr = stp.tile([128, 256], f32, tag="ysscr")
                    nc.vector.tensor_tensor(scr[:, :ms], pb[:, :ms],
                                            hs_p_full[:, o:o + ms], op=Mult)
                    acc = smallp.tile([128, 1], f32, tag="small")
                    nc.vector.reduce_sum(acc[:], scr[:, :ms], axis=AxX)
                    if prev is not None:
                        nc.vector.tensor_tensor(acc[:], acc[:], prev[:], op=Add)
                    prev = acc
                    o += ms
                # output MLP on cat = [compound, protein]
                cat = catp.tile([128, 2], f32, tag="cat")
                nc.vector.tensor_copy(cat[:, 0:1], comp[:])
                nc.vector.tensor_scalar_mul(cat[:, 1:2], prev[:], 1.0 / nw)
                for l in range(LAYER_OUT):
                    ncat = catp.tile([128, 2], f32, tag="cat")
                    for i in range(2):
                        pm = ps_misc.tile([128, 256], f32, tag="m256")
                        for j in range(2):
                            nc.tensor.matmul(
                                pm[:, :1],
                                woT_sb[:, (j * 2 + i) * DIM:(j * 2 + i + 1) * DIM],
                                cat[:, j:j + 1], start=(j == 0), stop=(j == 1))
                        nc.scalar.activation(ncat[:, i:i + 1], pm[:, :1], Relu,
                                             bias=bo_sb[:, i:i + 1])
                    cat = ncat
                pf = ps_misc.tile([128, 256], f32, tag="m256")
                for j in range(2):
                    nc.tensor.matmul(pf[:2, :1], wiT_sb[:, 2 * j:2 * j + 2],
                                     cat[:, j:j + 1], start=(j == 0), stop=(j == 1))
                res = smallp.tile([2, 1], f32, tag="res")
                nc.scalar.activation(res[:], pf[:2, :1], Ident, bias=bi_sb[:])
                nc.sync.dma_start(t_out[:], res[:])

            def finish_early():
                res2 = smallp.tile([2, 1], f32, tag="res")
                nc.vector.tensor_copy(res2[:], bi_sb[:])
                nc.sync.dma_start(t_out[:], res2[:])

            # ================= main body (reps > 1 for timing) =================
            for rep in range(reps):
                if stage < 2:
                    break
                # ---- GNN layer 0 SpMM; dot0 = cs.hs0 accumulated in-loop ----
                pd0 = ps_dot.tile([128, 128], f32, tag="pdot",
                                  name=f"pd0_r{rep}")

                def hs0_pair(kj, _pd0=pd0):
                    hp = ps_misc.tile([128, 256], f32, tag="m256")
                    hch = hsp.tile([128, 256], f8, tag="hs0c")
                    for i in range(2):
                        ki = 2 * kj + i
                        nc.tensor.matmul(hp[:, i * 128:(i + 1) * 128],
                                         ones_row[:], bg_bf[:], start=True,
                                         stop=False)
                        nc.tensor.matmul(hp[:, i * 128:(i + 1) * 128],
                                         xs0_full[:, ki * 128:(ki + 1) * 128],
                                         wgT_bf[:], start=False, stop=True)
                    nc.scalar.activation(hch[:], hp[:], Relu, scale=HS_SCALES[0])
                    for i in range(2):
                        ki = 2 * kj + i
                        nc.tensor.matmul(_pd0[:1, :128], csga_bf[:, ki:ki + 1],
                                         hch[:, i * 128:(i + 1) * 128],
                                         start=(ki == 0), stop=(ki == ach - 1))
                    return hch[:].rearrange("p (two d) -> p two d", two=2)

                psums0 = spmm_layer(0, rep, hs0_pair)
                xs1_c = xs_update(psums0, xs0_own, rep, 0)
                dot0_sb = dotp.tile([1, 128], f32, tag="dotrow")
                nc.scalar.activation(dot0_sb[:], pd0[:1, :128], Ident,
                                     scale=1.0 / HS_SCALES[0])
                if stage < 3:
                    continue
                # ---- own hs1; conv layer 1 fills the AllGather gap ----
                hso1 = own_hs(xs1_c, rep, 1)
                imgb = conv_layer(img0, 0, rep)
                if stage < 4:
                    continue
                hs1_full = allgather_hs(hso1, rep, 1)
                # protein side is GNN-independent: finish it in the AG gap
                agbt = agbp.tile([128, local_w + 2], bf16, tag="agbt",
                                 name=f"agbt_r{rep}")
                imgc = conv_layer(imgb, 1, rep)
                imgd = conv_layer(imgc, 2, rep)
                protein_hsp(imgd, agbt)
                # ---- GNN layer 1 SpMM; dot1 = cs.hs1 accumulated in-loop ----
                pd1 = ps_dot.tile([128, 128], f32, tag="pdot",
                                  name=f"pd1_r{rep}")

                def dot1_fn(kj, _pd1=pd1, _hs=hs1_full):
                    for i in range(2):
                        k = 2 * kj + i
                        nc.tensor.matmul(_pd1[:1, :128], csga_bf[:, k:k + 1],
                                         _hs[:, k * 128:(k + 1) * 128],
                                         start=(k == 0), stop=(k == ach - 1))

                hs1_r = hs1_full[:].rearrange("p (t d) -> p t d", d=128)
                psums1 = spmm_layer(1, rep,
                                    lambda kj: hs1_r[:, 2 * kj:2 * kj + 2, :],
                                    extra=dot1_fn)
                xs2_c = xs_update(psums1, xs1_c, rep, 1)
                dot1_sb = dotp.tile([1, 128], f32, tag="dotrow")
                nc.scalar.activation(dot1_sb[:], pd1[:1, :128], Ident,
                                     scale=1.0 / HS_SCALES[1])
                if stage < 5:
                    continue
                # ---- layer 2: hs2 own, dot2 partial, joint AllGather ----
                hso2 = own_hs(xs2_c, rep, 2)
                pd2 = ps_dot.tile([128, 128], f32, tag="pdot",
                                  name=f"pd2_r{rep}")
                for t in range(och):
                    nc.tensor.matmul(pd2[:1, :128], csgo_bf[:, t:t + 1],
                                     hso2[:, t * 128:(t + 1) * 128],
                                     start=(t == 0), stop=(t == och - 1))
                dot2_sb = dotp.tile([1, 128], f32, tag="dotrow")
                nc.scalar.activation(dot2_sb[:], pd2[:1, :128], Ident,
                                     scale=1.0 / HS_SCALES[2])
                pdt2 = ps_misc.tile([128, 256], f32, tag="m256")
                nc.tensor.matmul(pdt2[:, :1], dot2_sb[:], ones_f32[0:1, 0:1],
                                 start=True, stop=True)
                d2 = smallp.tile([128, 1], f32, tag="small")
                nc.vector.tensor_copy(d2[:], pdt2[:, :1])
                # hi/lo bf16 split of the f32 partial rides the AllGather
                nc.vector.tensor_copy(agbt[:, local_w:local_w + 1], d2[:])
                lo = smallp.tile([128, 1], f32, tag="small")
                nc.vector.tensor_tensor(lo[:], d2[:],
                                        agbt[:, local_w:local_w + 1], op=Sub)
                nc.vector.tensor_copy(agbt[:, local_w + 1:local_w + 2], lo[:])
                agbi = dram.tile([128, local_w + 2], bf16, tag=f"agbi_r{rep}",
                                 name=f"agbi_r{rep}")
                agbo = dram.tile([128 * ncores, local_w + 2], bf16,
                                 tag=f"agbo_r{rep}", name=f"agbo_r{rep}",
                                 addr_space="Shared")
                nc.gpsimd.dma_start(agbi[:], agbt[:])
                nc.gpsimd.collective_compute(
                    "AllGather", mybir.AluOpType.bypass,
                    ins=[agbi[:].opt()], outs=[agbo[:].opt()], replica_groups=rg)
                hs_p_full = hsfp.tile([128, nw], bf16, tag="hspfull",
                                      name=f"hspf_r{rep}")
                dots16 = smallp.tile([128, 2 * ncores], bf16, tag="dots16")
                for c in range(ncores):
                    nc.gpsimd.dma_start(
                        hs_p_full[:, c * local_w:(c + 1) * local_w],
                        agbo[c * 128:(c + 1) * 128, :local_w])
                    nc.gpsimd.dma_start(
                        dots16[:, 2 * c:2 * c + 2],
                        agbo[c * 128:(c + 1) * 128, local_w:local_w + 2])
                if stage < 7:
                    continue
                tail(rep, dot0_sb, dot1_sb, dots16, hs_p_full)

            if stage < 7:
                finish_early()

    nc.compile()
    return nc


def prep_in_maps(inputs, na, nw, nfp, nword, ncores):
    """Host-side sharding/layout prep (gathers + casts + transposes)."""
    local_a = na // ncores
    local_w = nw // ncores
    lwin = local_w + 2 * HALO
    wch = _ceil_div(lwin, 128)
    wpad = wch * 128

    fingerprints = np.asarray(inputs["fingerprints"]).astype(np.int64)
    adjacency = np.asarray(inputs["adjacency"], dtype=np.float32)
    words = np.asarray(inputs["words"]).astype(np.int64)
    embed_fp = np.asarray(inputs["embed_fp"], dtype=np.float32)
    embed_word = np.asarray(inputs["embed_word"], dtype=np.float32)
    Wg = np.asarray(inputs["Wg"], dtype=np.float32)
    bg = np.asarray(inputs["bg"], dtype=np.float32)
    conv_k = np.asarray(inputs["conv_k"], dtype=np.float32)
    conv_b = np.asarray(inputs["conv_b"], dtype=np.float32)
    Wa = np.asarray(inputs["Wa"], dtype=np.float32)
    ba = np.asarray(inputs["ba"], dtype=np.float32)
    Wo = np.asarray(inputs["Wo"], dtype=np.float32)
    bo = np.asarray(inputs["bo"], dtype=np.float32)
    Wi = np.asarray(inputs["Wi"], dtype=np.float32)
    bi = np.asarray(inputs["bi"], dtype=np.float32)

    # adjacency -> fp8 bit pattern (0.0 -> 0x00, nonzero -> 0x38 = 1.0 in e4m3)
    nz = adjacency != 0
    a8 = nz.astype(np.uint8) * np.uint8(0x38)
    colsum_g = nz.sum(axis=0).astype(np.float32)               # [na]

    # xs0 gather, dim-major, bf16 (replicated)
    xs0 = embed_fp[fingerprints]                       # [na, DIM] f32
    xs0T = np.ascontiguousarray(xs0.T).astype(BF16)    # [128, na]

    K2 = conv_k[0, 0]
    M = np.zeros((DIM, KK * DIM), np.float32)
    for a in range(KK):
        Ma = np.zeros((DIM, DIM), np.float32)
        for b_ in range(KK):
            Ma += K2[a, b_] * np.eye(DIM, k=5 - b_, dtype=np.float32)
        M[:, a * DIM:(a + 1) * DIM] = Ma

    # packf32 [128, 922]: layouts the kernel consumes directly
    PF = 922
    pf = np.zeros((DIM, PF), np.float32)
    pf[:, 0:128] = Wg.T
    WoT = Wo.T.astype(np.float32)
    WiT = Wi.T.astype(np.float32)
    for j in range(2):
        for i in range(2):
            pf[:, 128 + (j * 2 + i) * DIM:128 + (j * 2 + i + 1) * DIM] = \
                WoT[j * DIM:(j + 1) * DIM, i * DIM:(i + 1) * DIM]
        pf[:, 640 + j] = bo[j * DIM:(j + 1) * DIM]
        pf[:, 642 + 2 * j:644 + 2 * j] = WiT[j * DIM:(j + 1) * DIM, :]
    pf[:, 646] = ba
    pf[:, 647] = conv_b[0]
    pf[0:2, 648] = bi
    pf[0, 649:777] = bg
    pf[:, 777:905] = colsum_g.reshape(na // 128, 128).T
    pf[:, 921] = xs0.mean(axis=0)

    ws_full = embed_word[words]                        # [nw, DIM] f32

    PBF = 1536 + na + local_a + 2 * wpad
    ob_xs0own = 1536 + na
    ob_img0 = ob_xs0own + local_a
    ob_wmask = ob_img0 + wpad

    in_maps = []
    for c in range(ncores):
        sl = slice(c * local_a, (c + 1) * local_a)
        bmat = np.ascontiguousarray(a8[sl, :].T).view(F8)
        w0 = c * local_w - HALO
        pos = np.arange(wpad)
        gidx = w0 + pos
        valid = (gidx >= 0) & (gidx < nw) & (pos < lwin)
        win = np.where(valid[:, None], ws_full[np.clip(gidx, 0, nw - 1)], 0.0)
        pb = np.zeros((DIM, PBF), BF16)
        pb[:, 0:128] = Wa.T.astype(BF16)
        pb[:, 128:1536] = M.astype(BF16)
        pb[:, 1536:1536 + na] = xs0T
        pb[:, ob_xs0own:ob_xs0own + local_a] = xs0T[:, sl]
        pb[:, ob_img0:ob_img0 + wpad] = win.T.astype(BF16)
        pb[:, ob_wmask:ob_wmask + wpad] = \
            ((gidx >= 0) & (gidx < nw)).astype(BF16)[None, :]
        pfc = pf.copy()
        pfc[:, 905:921] = colsum_g[sl].reshape(local_a // 128, 128).T
        in_maps.append(dict(bmat=bmat, packbf=pb, packf32=pfc))
    return in_maps


_CACHE = {}


def _get_kernel(cfg_key):
    if cfg_key not in _CACHE:
        na, nw, nfp, nword, ncores = cfg_key
        _CACHE[cfg_key] = build_kernel(na, nw, nfp, nword, ncores)
    return _CACHE[cfg_key]


def kernel(**inputs) -> np.ndarray:
    from concourse import bass_utils
    cfg = FULL
    key = (cfg["na"], cfg["nw"], cfg["nfp"], cfg["nword"], cfg["ncores"])
    nc = _get_kernel(key)
    in_maps = prep_in_maps(inputs, *key)
    res = bass_utils.run_bass_kernel_spmd(
        nc, in_maps, core_ids=list(range(cfg["ncores"])), trace=False)
    out = np.asarray(res.results[0]["out"], np.float32).reshape(1, 2)
    return out


# revision 20
# speedup vs baseline: 1.0869x; 1.0869x over previous
"""CompoundProteinInteractionPrediction on 8 Trainium2 NeuronCores (Bass/Tile).

v3 restructure (vs v2):
- Inputs packed into 3 tensors (bmat / packbf / packf32) — per-arg dispatch
  cost through the axon client is significant.
- compound = mean(xs3) computed via host-side global colsum of A:
  mean(xs3) = mean(xs0) + (cs.hs0 + cs.hs1 + cs.hs2)/N. dot0/dot1 are
  computed locally on every core (hs0 from replicated xs0; hs1 post
  AllGather), accumulated on PE during the SpMM loops. Only dot2 needs a
  cross-core reduction.
- The protein attention features hs_p (independent of the GNN) and the
  dot2 partial (hi/lo bf16 split for ~f32 accuracy) ride ONE AllGather;
  each core then computes the exact tanh attention + output MLP locally.
  This removes v2's ReduceScatter and both tail AllReduces — per rep only
  two collectives remain (AG hs1, AG hs_p|dot2), both substantially
  overlapped with SpMM/conv work.
- reps>1 runs the whole model body repeatedly inside one NEFF for
  steady-state throughput timing (amortizes per-execution launch costs).

Engine assignment: slab DMAs on SP/ACT queues (alternating), collective
input + readback DMAs on the gpsimd queue so slab prefetch is never queued
behind a collective-blocked dma_start.
"""
import sys

sys.path.insert(0, "/opt/trn_rl_repo")

import numpy as np
import ml_dtypes

import concourse.bass as bass
import concourse.tile as tile
from concourse import bacc, mybir

F8 = ml_dtypes.float8_e4m3
BF16 = ml_dtypes.bfloat16

DIM = 128
WINDOW = 5
KK = 2 * WINDOW + 1
LAYER_GNN = 3
LAYER_CNN = 3
LAYER_OUT = 2
HALO = WINDOW * LAYER_CNN  # 15
HS_SCALES = (1024.0, 128.0, 16.0)  # per-layer fp8 scale for hs (max ~140-165)
HSP_SCALE = 8192.0   # fp8 scale for protein hs_p (max ~0.013)

FULL = dict(na=16384, nw=16384, nfp=100000, nword=20000, ncores=8)


def _ceil_div(a, b):
    return (a + b - 1) // b


def build_kernel(na, nw, nfp, nword, ncores, reps=1, stage=7,
                 dma_mode="alt", slab_bufs=8, cache_pairs=6):
    local_a = na // ncores
    local_w = nw // ncores
    ach = na // 128            # 128 atom chunks of 128
    och = local_a // 128       # 16 own chunks per core
    lwin = local_w + 2 * HALO
    wch = _ceil_div(lwin, 128)
    wpad = wch * 128
    n_mb = local_a // 512      # 4 psum column tiles of 512

    f32 = mybir.dt.float32
    bf16 = mybir.dt.bfloat16
    f8 = mybir.dt.float8e4
    Relu = mybir.ActivationFunctionType.Relu
    Tanh = mybir.ActivationFunctionType.Tanh
    Ident = mybir.ActivationFunctionType.Identity
    DR = mybir.MatmulPerfMode.DoubleRow
    Add = mybir.AluOpType.add
    Sub = mybir.AluOpType.subtract
    Mult = mybir.AluOpType.mult
    AxX = mybir.AxisListType.X

    nc = bacc.Bacc("TRN2", target_bir_lowering=False, debug=False,
                   enable_asserts=False, num_devices=ncores)

    # ---- DRAM inputs (per-core values via in_maps) ----
    # packf32 columns: [wgT 0:128 | woT blocks 128:640 | bo 640:642 |
    #   wiT 642:646 | ba 646 | convb 647 | bi(rows 0:2) 648 | bg(row 0)
    #   649:777 | csg_all 777:905 | csg_own 905:921 | mean_xs0 921]
    # packbf columns: [waT 0:128 | convm 128:1536 | xs0T 1536:1536+na |
    #   xs0ownT +local_a | img0 +wpad | wmask +wpad]
    PF = 922
    PBF = 1536 + na + local_a + 2 * wpad
    ob_xs0own = 1536 + na
    ob_img0 = ob_xs0own + local_a
    ob_wmask = ob_img0 + wpad
    t_bmat = nc.dram_tensor("bmat", [na, local_a], f8, kind="ExternalInput").ap()
    t_packbf = nc.dram_tensor("packbf", [128, PBF], bf16, kind="ExternalInput").ap()
    t_packf32 = nc.dram_tensor("packf32", [128, PF], f32, kind="ExternalInput").ap()
    t_out = nc.dram_tensor("out", [2, 1], f32, kind="ExternalOutput").ap()

    rg = [list(range(ncores))]
    bmat_r = t_bmat.rearrange("(t p) m -> t p m", p=128)
    bmat_p = t_bmat.rearrange("(t two p) m -> t p two m", two=2, p=128)

    with tile.TileContext(nc) as tc:
        with (
            tc.tile_pool(name="persist", bufs=1) as persist,
            tc.tile_pool(name="hsp", bufs=4) as hsp,
            tc.tile_pool(name="slabp", bufs=slab_bufs) as slabp,
            tc.tile_pool(name="xsp", bufs=2) as xsp,
            tc.tile_pool(name="convp", bufs=3) as convp,
            tc.tile_pool(name="hsfp", bufs=1) as hsfp,
            tc.tile_pool(name="agbp", bufs=2) as agbp,
            tc.tile_pool(name="stp", bufs=3) as stp,
            tc.tile_pool(name="smallp", bufs=12) as smallp,
            tc.tile_pool(name="dotp", bufs=3) as dotp,
            tc.tile_pool(name="catp", bufs=3) as catp,
            tc.tile_pool(name="dram", bufs=1, space="DRAM") as dram,
            tc.tile_pool(name="ps_spmm", bufs=1, space="PSUM") as ps_spmm,
            tc.tile_pool(name="ps_misc", bufs=2, space="PSUM") as ps_misc,
            tc.tile_pool(name="ps_conv", bufs=1, space="PSUM") as ps_conv,
            tc.tile_pool(name="ps_dot", bufs=1, space="PSUM") as ps_dot,
        ):
            # ---- persistent small weights ----
            wgT = persist.tile([DIM, DIM], f32, tag="wgT")
            wgT_bf = persist.tile([DIM, DIM], bf16, tag="wgT_bf")
            bg_row = persist.tile([1, DIM], f32, tag="bg_row")
            waT = persist.tile([DIM, DIM], bf16, tag="waT")
            ba_col = persist.tile([DIM, 1], f32, tag="ba_col")
            convm = persist.tile([DIM, KK * DIM], bf16, tag="convm")
            convb_col = persist.tile([DIM, 1], f32, tag="convb_col")
            woT_sb = persist.tile([DIM, 4 * DIM], f32, tag="woT_sb")
            bo_sb = persist.tile([DIM, 2], f32, tag="bo_sb")
            wiT_sb = persist.tile([DIM, 4], f32, tag="wiT_sb")
            bi_sb = persist.tile([2, 1], f32, tag="bi_sb")
            ones_row = persist.tile([1, DIM], bf16, tag="ones_row")
            ones_f32 = persist.tile([1, DIM], f32, tag="ones_f32")
            bg_bf = persist.tile([1, DIM], bf16, tag="bg_bf")
            xs0_full = persist.tile([128, na], bf16, tag="xs0_full")
            xs0_own = persist.tile([128, local_a], bf16, tag="xs0_own")
            img0 = persist.tile([128, wpad], bf16, tag="img0")
            wmask_sb = persist.tile([128, wpad], bf16, tag="wmask_sb")
            csga_f32 = persist.tile([DIM, DIM], f32, tag="csga_f32")
            csga_bf = persist.tile([DIM, DIM], bf16, tag="csga_bf")
            csgo_f32 = persist.tile([DIM, och], f32, tag="csgo_f32")
            csgo_bf = persist.tile([DIM, och], bf16, tag="csgo_bf")
            mean_xs0 = persist.tile([DIM, 1], f32, tag="mean_xs0")
            ba_hsp = persist.tile([DIM, 1], f32, tag="ba_hsp")
            bcache = (persist.tile([128, cache_pairs * 2 * local_a], f8,
                                   tag="bcache", name="bcache") if cache_pairs else None)

            nc.sync.dma_start(wgT[:], t_packf32[:, 0:128])
            nc.vector.tensor_copy(wgT_bf[:], wgT[:])
            nc.sync.dma_start(bg_row[:], t_packf32[0:1, 649:777])
            nc.sync.dma_start(waT[:], t_packbf[:, 0:128])
            nc.sync.dma_start(ba_col[:], t_packf32[:, 646:647])
            nc.sync.dma_start(convm[:], t_packbf[:, 128:1536])
            nc.sync.dma_start(convb_col[:], t_packf32[:, 647:648])
            nc.sync.dma_start(woT_sb[:], t_packf32[:, 128:640])
            nc.sync.dma_start(bo_sb[:], t_packf32[:, 640:642])
            nc.sync.dma_start(wiT_sb[:], t_packf32[:, 642:646])
            nc.sync.dma_start(bi_sb[:], t_packf32[0:2, 648:649])
            nc.sync.dma_start(csga_f32[:], t_packf32[:, 777:905])
            nc.vector.tensor_copy(csga_bf[:], csga_f32[:])
            nc.sync.dma_start(csgo_f32[:], t_packf32[:, 905:921])
            nc.vector.tensor_copy(csgo_bf[:], csgo_f32[:])
            nc.sync.dma_start(mean_xs0[:], t_packf32[:, 921:922])
            nc.vector.tensor_scalar_mul(ba_hsp[:], ba_col[:], HSP_SCALE)
            nc.gpsimd.memset(ones_f32[:], 1.0)
            nc.vector.tensor_copy(ones_row[:], ones_f32[:])
            nc.vector.tensor_copy(bg_bf[:], bg_row[:])
            nc.sync.dma_start(xs0_full[:], t_packbf[:, 1536:1536 + na])
            nc.sync.dma_start(xs0_own[:],
                              t_packbf[:, ob_xs0own:ob_xs0own + local_a])
            nc.sync.dma_start(img0[:], t_packbf[:, ob_img0:ob_img0 + wpad])
            nc.sync.dma_start(wmask_sb[:], t_packbf[:, ob_wmask:ob_wmask + wpad])

            def spmm_layer(layer, rep, hs_src, extra=None):
                """SpMM over own rows via fp8 DoubleRow; psum [dims, local_a].
                hs_src(kj) -> lhsT AP [128, 2, DIM] f8 for chunk pair kj.
                extra(kj), if given, is emitted once per pair (dot matmuls)."""
                psums = [ps_spmm.tile([128, 512], f32, tag=f"spmm{mb}",
                                      name=f"spmm_r{rep}_l{layer}_{mb}")
                         for mb in range(n_mb)]
                kc0 = ach // 2 - cache_pairs
                for kj in range(ach // 2):
                    if cache_pairs and kj >= kc0:
                        slab = bcache[:, (kj - kc0) * 2 * local_a:
                                      (kj - kc0 + 1) * 2 * local_a]
                        if layer == 0:
                            eng = nc.sync if kj % 2 == 0 else nc.scalar
                            if dma_mode == "pair1":
                                eng.dma_start(
                                    slab.rearrange("p (two m) -> p two m", two=2),
                                    bmat_p[kj])
                            else:
                                eng.dma_start(slab[:, :local_a], bmat_r[2 * kj])
                                eng.dma_start(slab[:, local_a:], bmat_r[2 * kj + 1])
                        slab3 = slab.rearrange("p (two m) -> p two m", two=2)
                    else:
                        slab = slabp.tile([128, 2 * local_a], f8, tag="slab")
                        if dma_mode == "pair1":
                            eng = nc.sync if kj % 2 == 0 else nc.scalar
                            eng.dma_start(
                                slab[:].rearrange("p (two m) -> p two m", two=2),
                                bmat_p[kj])
                        elif dma_mode == "sp":
                            nc.sync.dma_start(slab[:, :local_a], bmat_r[2 * kj])
                            nc.sync.dma_start(slab[:, local_a:], bmat_r[2 * kj + 1])
                        elif dma_mode == "alt":
                            eng = nc.sync if kj % 2 == 0 else nc.scalar
                            eng.dma_start(slab[:, :local_a], bmat_r[2 * kj])
                            eng.dma_start(slab[:, local_a:], bmat_r[2 * kj + 1])
                        else:  # "half": one half per engine every pair
                            nc.sync.dma_start(slab[:, :local_a], bmat_r[2 * kj])
                            nc.scalar.dma_start(slab[:, local_a:], bmat_r[2 * kj + 1])
                        slab3 = slab[:].rearrange("p (two m) -> p two m", two=2)
                    lhsT = hs_src(kj)
                    for mb in range(n_mb):
                        nc.tensor.matmul(
                            psums[mb][:], lhsT, slab3[:, :, mb * 512:(mb + 1) * 512],
                            start=(kj == 0), stop=(kj == ach // 2 - 1),
                            perf_mode=DR)
                    if extra is not None:
                        extra(kj)
                return psums

            def own_hs(xs_c, rep, layer):
                """hs (fp8, scaled) for own atoms from xs_c [128(dim), local_a]."""
                hso = hsp.tile([128, och * 128], f8, tag="hso",
                               name=f"hso_r{rep}_l{layer}")
                for t in range(och):
                    hp = ps_misc.tile([128, 256], f32, tag="m256")
                    nc.tensor.matmul(hp[:, :128], ones_f32[:], bg_row[:],
                                     start=True, stop=False)
                    nc.tensor.matmul(hp[:, :128], xs_c[:, t * 128:(t + 1) * 128],
                                     wgT[:], start=False, stop=True)
                    nc.scalar.activation(hso[:, t * 128:(t + 1) * 128], hp[:, :128],
                                         Relu, scale=HS_SCALES[layer])
                return hso

            def allgather_hs(hso, rep, layer):
                """own hs f8 [128, 2048] -> hs_full f8 [128, ach*128].
                All DMAs on gpsimd queue (keeps SP/ACT slab prefetch clear)."""
                agi = dram.tile([128, och * 128], f8, tag=f"agi_r{rep}_l{layer}",
                                name=f"agi_r{rep}_l{layer}")
                ago = dram.tile([128 * ncores, och * 128], f8,
                                tag=f"ago_r{rep}_l{layer}",
                                name=f"ago_r{rep}_l{layer}", addr_space="Shared")
                nc.gpsimd.dma_start(agi[:], hso[:])
                nc.gpsimd.collective_compute(
                    "AllGather", mybir.AluOpType.bypass,
                    ins=[agi[:].opt()], outs=[ago[:].opt()], replica_groups=rg)
                hs_full = hsfp.tile([128, ach * 128], f8, tag="hsfull",
                                    name=f"hsfull_r{rep}_l{layer}")
                for c in range(ncores):
                    nc.gpsimd.dma_start(
                        hs_full[:, c * och * 128:(c + 1) * och * 128],
                        ago[c * 128:(c + 1) * 128, :])
                return hs_full

            def xs_update(psums, base_bf, rep, layer):
                """xs_new f32 = base + psum/HS_SCALES[layer]."""
                xs_c = xsp.tile([128, local_a], f32, tag="xs_c",
                                name=f"xs_r{rep}_l{layer}")
                for mb in range(n_mb):
                    nc.vector.scalar_tensor_tensor(
                        xs_c[:, mb * 512:(mb + 1) * 512], psums[mb][:],
                        1.0 / HS_SCALES[layer], base_bf[:, mb * 512:(mb + 1) * 512],
                        op0=Mult, op1=Add)
                return xs_c

            # ---------------- protein conv pieces ----------------
            def conv_layer(src, l, rep):
                lo = WINDOW * (l + 1)
                hi = lwin - WINDOW * (l + 1)
                dst = convp.tile([128, wpad], bf16, tag="convb",
                                 name=f"conv_r{rep}_l{l}")
                o = lo
                while o < hi:
                    ms = min(512, hi - o)
                    pc = ps_conv.tile([128, 512], f32, tag="pconv")
                    for a in range(KK):
                        nc.tensor.matmul(
                            pc[:, :ms], convm[:, a * DIM:(a + 1) * DIM],
                            src[:, o + a - WINDOW:o + a - WINDOW + ms],
                            start=(a == 0), stop=(a == KK - 1))
                    nc.scalar.activation(dst[:, o:o + ms], pc[:, :ms], Relu,
                                         bias=convb_col[:])
                    o += ms
                if l < LAYER_CNN - 1:
                    nc.vector.tensor_tensor(dst[:, lo:hi], dst[:, lo:hi],
                                            wmask_sb[:, lo:hi], op=Mult)
                return dst

            def protein_hsp(xsp_img, dst):
                """hs_p = relu(Wa xs_p + ba) for own words -> dst[:, :local_w]."""
                o = 0
                while o < local_w:
                    ms = min(512, local_w - o)
                    pc = ps_conv.tile([128, 512], f32, tag="pconv")
                    nc.tensor.matmul(pc[:, :ms], waT[:],
                                     xsp_img[:, HALO + o:HALO + o + ms],
                                     start=True, stop=True)
                    nc.scalar.activation(dst[:, o:o + ms], pc[:, :ms], Relu,
                                         scale=HSP_SCALE, bias=ba_hsp[:])
                    o += ms

            def tail(rep, dot0_sb, dot1_sb, dots8, hs_p_full):
                # dot2 = sum of gathered f32 partials
                d2s = smallp.tile([128, 1], f32, tag="small")
                nc.vector.reduce_sum(d2s[:], dots8[:], axis=AxX)
                # transpose dot0+dot1 rows -> [128,1] column, accumulated
                pdt = ps_misc.tile([128, 256], f32, tag="m256")
                nc.tensor.matmul(pdt[:, :1], dot0_sb[:], ones_f32[0:1, 0:1],
                                 start=True, stop=False)
                nc.tensor.matmul(pdt[:, :1], dot1_sb[:], ones_f32[0:1, 0:1],
                                 start=False, stop=True)
                t1 = smallp.tile([128, 1], f32, tag="small")
                nc.vector.tensor_tensor(t1[:], d2s[:], pdt[:, :1], op=Add)
                comp = smallp.tile([128, 1], f32, tag="small")
                nc.vector.scalar_tensor_tensor(comp[:], t1[:], 1.0 / na,
                                               mean_xs0[:], op0=Mult, op1=Add)
                # h = relu(Wa compound + ba)
                comp_bf = smallp.tile([128, 1], bf16, tag="smallbf")
                nc.vector.tensor_copy(comp_bf[:], comp[:])
                ph = ps_conv.tile([128, 512], f32, tag="pconv")
                nc.tensor.matmul(ph[:, :1], waT[:], comp_bf[:], start=True,
                                 stop=True)
                h_bf = smallp.tile([128, 1], bf16, tag="smallbf")
                nc.scalar.activation(h_bf[:], ph[:, :1], Relu, bias=ba_col[:])
                # exact attention over ALL nw words (hs_p_full gathered, fp8
                # scaled by HSP_SCALE; tanh arg rescaled inside activation)
                prev = None
                o = 0
                while o < nw:
                    ms = min(512, nw - o)
                    pw = ps_dot.tile([128, 512], f32, tag="pdot")
                    nc.tensor.matmul(pw[:1, :ms], h_bf[:],
                                     hs_p_full[:, o:o + ms], start=True, stop=True)
                    w_bf = stp.tile([1, 512], bf16, tag="wrow")
                    nc.scalar.activation(w_bf[:, :ms], pw[:1, :ms], Tanh,
                                         scale=1.0 / HSP_SCALE)
                    pb = ps_conv.tile([128, 512], f32, tag="pconv")
                    nc.tensor.matmul(pb[:, :ms], ones_row[:], w_bf[:, :ms],
                                     start=True, stop=True)
                    scr = stp.tile([128, 512], f32, tag="ysscr")
                    nc.vector.tensor_tensor(scr[:, :ms], pb[:, :ms],
                                            hs_p_full[:, o:o + ms], op=Mult)
                    acc = smallp.tile([128, 1], f32, tag="small")
                    nc.vector.reduce_sum(acc[:], scr[:, :ms], axis=AxX)
                    if prev is not None:
                        nc.vector.tensor_tensor(acc[:], acc[:], prev[:], op=Add)
                    prev = acc
                    o += ms
                # output MLP on cat = [compound, protein]
                cat = catp.tile([128, 2], f32, tag="cat")
                nc.vector.tensor_copy(cat[:, 0:1], comp[:])
                nc.vector.tensor_scalar_mul(cat[:, 1:2], prev[:],
                                            1.0 / (nw * HSP_SCALE))
                for l in range(LAYER_OUT):
                    ncat = catp.tile([128, 2], f32, tag="cat")
                    for i in range(2):
                        pm = ps_misc.tile([128, 256], f32, tag="m256")
                        for j in range(2):
                            nc.tensor.matmul(
                                pm[:, :1],
                                woT_sb[:, (j * 2 + i) * DIM:(j * 2 + i + 1) * DIM],
                                cat[:, j:j + 1], start=(j == 0), stop=(j == 1))
                        nc.scalar.activation(ncat[:, i:i + 1], pm[:, :1], Relu,
                                             bias=bo_sb[:, i:i + 1])
                    cat = ncat
                pf = ps_misc.tile([128, 256], f32, tag="m256")
                for j in range(2):
                    nc.tensor.matmul(pf[:2, :1], wiT_sb[:, 2 * j:2 * j + 2],
                                     cat[:, j:j + 1], start=(j == 0), stop=(j == 1))
                res = smallp.tile([2, 1], f32, tag="res")
                nc.scalar.activation(res[:], pf[:2, :1], Ident, bias=bi_sb[:])
                nc.sync.dma_start(t_out[:], res[:])

            def finish_early():
                res2 = smallp.tile([2, 1], f32, tag="res")
                nc.vector.tensor_copy(res2[:], bi_sb[:])
                nc.sync.dma_start(t_out[:], res2[:])

            # ================= main body (reps > 1 for timing) =================
            for rep in range(reps):
                if stage < 2:
                    break
                # ---- GNN layer 0 SpMM; dot0 = cs.hs0 accumulated in-loop ----
                pd0 = ps_dot.tile([128, 128], f32, tag="pdot",
                                  name=f"pd0_r{rep}")

                def hs0_pair(kj, _pd0=pd0):
                    hp = ps_misc.tile([128, 256], f32, tag="m256")
                    hch = hsp.tile([128, 256], f8, tag="hs0c")
                    for i in range(2):
                        ki = 2 * kj + i
                        nc.tensor.matmul(hp[:, i * 128:(i + 1) * 128],
                                         ones_row[:], bg_bf[:], start=True,
                                         stop=False)
                        nc.tensor.matmul(hp[:, i * 128:(i + 1) * 128],
                                         xs0_full[:, ki * 128:(ki + 1) * 128],
                                         wgT_bf[:], start=False, stop=True)
                    nc.scalar.activation(hch[:], hp[:], Relu, scale=HS_SCALES[0])
                    for i in range(2):
                        ki = 2 * kj + i
                        nc.tensor.matmul(_pd0[:1, :128], csga_bf[:, ki:ki + 1],
                                         hch[:, i * 128:(i + 1) * 128],
                                         start=(ki == 0), stop=(ki == ach - 1))
                    return hch[:].rearrange("p (two d) -> p two d", two=2)

                psums0 = spmm_layer(0, rep, hs0_pair)
                xs1_c = xs_update(psums0, xs0_own, rep, 0)
                dot0_sb = dotp.tile([1, 128], f32, tag="dotrow")
                nc.scalar.activation(dot0_sb[:], pd0[:1, :128], Ident,
                                     scale=1.0 / HS_SCALES[0])
                if stage < 3:
                    continue
                # ---- own hs1; conv layer 1 fills the AllGather gap ----
                hso1 = own_hs(xs1_c, rep, 1)
                imgb = conv_layer(img0, 0, rep)
                if stage < 4:
                    continue
                hs1_full = allgather_hs(hso1, rep, 1)
                # protein side is GNN-independent: finish it in the AG gap
                agbt = agbp.tile([128, local_w], f8, tag="agbt",
                                 name=f"agbt_r{rep}")
                imgc = conv_layer(imgb, 1, rep)
                imgd = conv_layer(imgc, 2, rep)
                protein_hsp(imgd, agbt)
                # AllGather hs_p NOW — it hides under the layer-1 SpMM
                agbi = dram.tile([128, local_w], f8, tag=f"agbi_r{rep}",
                                 name=f"agbi_r{rep}")
                agbo = dram.tile([128 * ncores, local_w], f8,
                                 tag=f"agbo_r{rep}", name=f"agbo_r{rep}",
                                 addr_space="Shared")
                nc.gpsimd.dma_start(agbi[:], agbt[:])
                nc.gpsimd.collective_compute(
                    "AllGather", mybir.AluOpType.bypass,
                    ins=[agbi[:].opt()], outs=[agbo[:].opt()], replica_groups=rg)
                hs_p_full = hsfp.tile([128, nw], f8, tag="hspfull",
                                      name=f"hspf_r{rep}")
                for c in range(ncores):
                    eng = (nc.gpsimd, nc.sync, nc.scalar)[c % 3]
                    eng.dma_start(
                        hs_p_full[:, c * local_w:(c + 1) * local_w],
                        agbo[c * 128:(c + 1) * 128, :])
                # ---- GNN layer 1 SpMM; dot1 = cs.hs1 accumulated in-loop ----
                pd1 = ps_dot.tile([128, 128], f32, tag="pdot",
                                  name=f"pd1_r{rep}")

                def dot1_fn(kj, _pd1=pd1, _hs=hs1_full):
                    for i in range(2):
                        k = 2 * kj + i
                        nc.tensor.matmul(_pd1[:1, :128], csga_bf[:, k:k + 1],
                                         _hs[:, k * 128:(k + 1) * 128],
                                         start=(k == 0), stop=(k == ach - 1))

                hs1_r = hs1_full[:].rearrange("p (t d) -> p t d", d=128)
                psums1 = spmm_layer(1, rep,
                                    lambda kj: hs1_r[:, 2 * kj:2 * kj + 2, :],
                                    extra=dot1_fn)
                xs2_c = xs_update(psums1, xs1_c, rep, 1)
                dot1_sb = dotp.tile([1, 128], f32, tag="dotrow")
                nc.scalar.activation(dot1_sb[:], pd1[:1, :128], Ident,
                                     scale=1.0 / HS_SCALES[1])
                if stage < 5:
                    continue
                # ---- layer 2: hs2 own, dot2 partial, joint AllGather ----
                hso2 = own_hs(xs2_c, rep, 2)
                pd2 = ps_dot.tile([128, 128], f32, tag="pdot",
                                  name=f"pd2_r{rep}")
                for t in range(och):
                    nc.tensor.matmul(pd2[:1, :128], csgo_bf[:, t:t + 1],
                                     hso2[:, t * 128:(t + 1) * 128],
                                     start=(t == 0), stop=(t == och - 1))
                dot2_sb = dotp.tile([1, 128], f32, tag="dotrow")
                nc.scalar.activation(dot2_sb[:], pd2[:1, :128], Ident,
                                     scale=1.0 / HS_SCALES[2])
                pdt2 = ps_misc.tile([128, 256], f32, tag="m256")
                nc.tensor.matmul(pdt2[:, :1], dot2_sb[:], ones_f32[0:1, 0:1],
                                 start=True, stop=True)
                d2 = smallp.tile([128, 1], f32, tag="small")
                nc.vector.tensor_copy(d2[:], pdt2[:, :1])
                # tiny exact f32 AllGather of the dot2 partials
                agci = dram.tile([128, 1], f32, tag=f"agci_r{rep}",
                                 name=f"agci_r{rep}")
                agco = dram.tile([128 * ncores, 1], f32, tag=f"agco_r{rep}",
                                 name=f"agco_r{rep}", addr_space="Shared")
                nc.gpsimd.dma_start(agci[:], d2[:])
                nc.gpsimd.collective_compute(
                    "AllGather", mybir.AluOpType.bypass,
                    ins=[agci[:].opt()], outs=[agco[:].opt()], replica_groups=rg)
                dots8 = smallp.tile([128, ncores], f32, tag="dots8")
                for c in range(ncores):
                    nc.gpsimd.dma_start(dots8[:, c:c + 1],
                                        agco[c * 128:(c + 1) * 128, :])
                if stage < 7:
                    continue
                tail(rep, dot0_sb, dot1_sb, dots8, hs_p_full)

            if stage < 7:
                finish_early()

    nc.compile()
    return nc


def prep_in_maps(inputs, na, nw, nfp, nword, ncores):
    """Host-side sharding/layout prep (gathers + casts + transposes)."""
    local_a = na // ncores
    local_w = nw // ncores
    lwin = local_w + 2 * HALO
    wch = _ceil_div(lwin, 128)
    wpad = wch * 128

    fingerprints = np.asarray(inputs["fingerprints"]).astype(np.int64)
    adjacency = np.asarray(inputs["adjacency"], dtype=np.float32)
    words = np.asarray(inputs["words"]).astype(np.int64)
    embed_fp = np.asarray(inputs["embed_fp"], dtype=np.float32)
    embed_word = np.asarray(inputs["embed_word"], dtype=np.float32)
    Wg = np.asarray(inputs["Wg"], dtype=np.float32)
    bg = np.asarray(inputs["bg"], dtype=np.float32)
    conv_k = np.asarray(inputs["conv_k"], dtype=np.float32)
    conv_b = np.asarray(inputs["conv_b"], dtype=np.float32)
    Wa = np.asarray(inputs["Wa"], dtype=np.float32)
    ba = np.asarray(inputs["ba"], dtype=np.float32)
    Wo = np.asarray(inputs["Wo"], dtype=np.float32)
    bo = np.asarray(inputs["bo"], dtype=np.float32)
    Wi = np.asarray(inputs["Wi"], dtype=np.float32)
    bi = np.asarray(inputs["bi"], dtype=np.float32)

    # adjacency -> fp8 bit pattern (0.0 -> 0x00, nonzero -> 0x38 = 1.0 in e4m3)
    nz = adjacency != 0
    a8 = nz.astype(np.uint8) * np.uint8(0x38)
    colsum_g = nz.sum(axis=0).astype(np.float32)               # [na]

    # xs0 gather, dim-major, bf16 (replicated)
    xs0 = embed_fp[fingerprints]                       # [na, DIM] f32
    xs0T = np.ascontiguousarray(xs0.T).astype(BF16)    # [128, na]

    K2 = conv_k[0, 0]
    M = np.zeros((DIM, KK * DIM), np.float32)
    for a in range(KK):
        Ma = np.zeros((DIM, DIM), np.float32)
        for b_ in range(KK):
            Ma += K2[a, b_] * np.eye(DIM, k=5 - b_, dtype=np.float32)
        M[:, a * DIM:(a + 1) * DIM] = Ma

    # packf32 [128, 922]: layouts the kernel consumes directly
    PF = 922
    pf = np.zeros((DIM, PF), np.float32)
    pf[:, 0:128] = Wg.T
    WoT = Wo.T.astype(np.float32)
    WiT = Wi.T.astype(np.float32)
    for j in range(2):
        for i in range(2):
            pf[:, 128 + (j * 2 + i) * DIM:128 + (j * 2 + i + 1) * DIM] = \
                WoT[j * DIM:(j + 1) * DIM, i * DIM:(i + 1) * DIM]
        pf[:, 640 + j] = bo[j * DIM:(j + 1) * DIM]
        pf[:, 642 + 2 * j:644 + 2 * j] = WiT[j * DIM:(j + 1) * DIM, :]
    pf[:, 646] = ba
    pf[:, 647] = conv_b[0]
    pf[0:2, 648] = bi
    pf[0, 649:777] = bg
    pf[:, 777:905] = colsum_g.reshape(na // 128, 128).T
    pf[:, 921] = xs0.mean(axis=0)

    ws_full = embed_word[words]                        # [nw, DIM] f32

    PBF = 1536 + na + local_a + 2 * wpad
    ob_xs0own = 1536 + na
    ob_img0 = ob_xs0own + local_a
    ob_wmask = ob_img0 + wpad

    in_maps = []
    for c in range(ncores):
        sl = slice(c * local_a, (c + 1) * local_a)
        bmat = np.ascontiguousarray(a8[sl, :].T).view(F8)
        w0 = c * local_w - HALO
        pos = np.arange(wpad)
        gidx = w0 + pos
        valid = (gidx >= 0) & (gidx < nw) & (pos < lwin)
        win = np.where(valid[:, None], ws_full[np.clip(gidx, 0, nw - 1)], 0.0)
        pb = np.zeros((DIM, PBF), BF16)
        pb[:, 0:128] = Wa.T.astype(BF16)
        pb[:, 128:1536] = M.astype(BF16)
        pb[:, 1536:1536 + na] = xs0T
        pb[:, ob_xs0own:ob_xs0own + local_a] = xs0T[:, sl]
        pb[:, ob_img0:ob_img0 + wpad] = win.T.astype(BF16)
        pb[:, ob_wmask:ob_wmask + wpad] = \
            ((gidx >= 0) & (gidx < nw)).astype(BF16)[None, :]
        pfc = pf.copy()
        pfc[:, 905:921] = colsum_g[sl].reshape(local_a // 128, 128).T
        in_maps.append(dict(bmat=bmat, packbf=pb, packf32=pfc))
    return in_maps


_CACHE = {}


def _get_kernel(cfg_key):
    if cfg_key not in _CACHE:
        na, nw, nfp, nword, ncores = cfg_key
        _CACHE[cfg_key] = build_kernel(na, nw, nfp, nword, ncores)
    return _CACHE[cfg_key]


def kernel(**inputs) -> np.ndarray:
    from concourse import bass_utils
    cfg = FULL
    key = (cfg["na"], cfg["nw"], cfg["nfp"], cfg["nword"], cfg["ncores"])
    nc = _get_kernel(key)
    in_maps = prep_in_maps(inputs, *key)
    res = bass_utils.run_bass_kernel_spmd(
        nc, in_maps, core_ids=list(range(cfg["ncores"])), trace=False)
    out = np.asarray(res.results[0]["out"], np.float32).reshape(1, 2)
    return out


# revision 23
# speedup vs baseline: 1.1557x; 1.0633x over previous
"""CompoundProteinInteractionPrediction on 8 Trainium2 NeuronCores (Bass/Tile).

v3 restructure (vs v2):
- Inputs packed into 3 tensors (bmat / packbf / packf32) — per-arg dispatch
  cost through the axon client is significant.
- compound = mean(xs3) computed via host-side global colsum of A:
  mean(xs3) = mean(xs0) + (cs.hs0 + cs.hs1 + cs.hs2)/N. dot0/dot1 are
  computed locally on every core (hs0 from replicated xs0; hs1 post
  AllGather), accumulated on PE during the SpMM loops. Only dot2 needs a
  cross-core reduction.
- The protein attention features hs_p (independent of the GNN) and the
  dot2 partial (hi/lo bf16 split for ~f32 accuracy) ride ONE AllGather;
  each core then computes the exact tanh attention + output MLP locally.
  This removes v2's ReduceScatter and both tail AllReduces — per rep only
  two collectives remain (AG hs1, AG hs_p|dot2), both substantially
  overlapped with SpMM/conv work.
- reps>1 runs the whole model body repeatedly inside one NEFF for
  steady-state throughput timing (amortizes per-execution launch costs).

Engine assignment: slab DMAs on SP/ACT queues (alternating), collective
input + readback DMAs on the gpsimd queue so slab prefetch is never queued
behind a collective-blocked dma_start.
"""
import sys

sys.path.insert(0, "/opt/trn_rl_repo")

import numpy as np
import ml_dtypes

import concourse.bass as bass
import concourse.tile as tile
from concourse import bacc, mybir

F8 = ml_dtypes.float8_e4m3
BF16 = ml_dtypes.bfloat16

DIM = 128
WINDOW = 5
KK = 2 * WINDOW + 1
LAYER_GNN = 3
LAYER_CNN = 3
LAYER_OUT = 2
HALO = WINDOW * LAYER_CNN  # 15
HS_SCALES = (1024.0, 128.0, 16.0)  # per-layer fp8 scale for hs (max ~140-165)
HSP_SCALE = 8192.0   # fp8 scale for protein hs_p (max ~0.013)

FULL = dict(na=16384, nw=16384, nfp=100000, nword=20000, ncores=8)


def _ceil_div(a, b):
    return (a + b - 1) // b


def build_kernel(na, nw, nfp, nword, ncores, reps=1, stage=7,
                 dma_mode="alt", slab_bufs=8, cache_pairs=6):
    local_a = na // ncores
    local_w = nw // ncores
    ach = na // 128            # 128 atom chunks of 128
    och = local_a // 128       # 16 own chunks per core
    lwin = local_w + 2 * HALO
    wch = _ceil_div(lwin, 128)
    wpad = wch * 128
    n_mb = local_a // 512      # 4 psum column tiles of 512

    f32 = mybir.dt.float32
    bf16 = mybir.dt.bfloat16
    f8 = mybir.dt.float8e4
    Relu = mybir.ActivationFunctionType.Relu
    Tanh = mybir.ActivationFunctionType.Tanh
    Ident = mybir.ActivationFunctionType.Identity
    DR = mybir.MatmulPerfMode.DoubleRow
    Add = mybir.AluOpType.add
    Sub = mybir.AluOpType.subtract
    Mult = mybir.AluOpType.mult
    AxX = mybir.AxisListType.X

    nc = bacc.Bacc("TRN2", target_bir_lowering=False, debug=False,
                   enable_asserts=False, num_devices=ncores)

    # ---- DRAM inputs (per-core values via in_maps) ----
    # packf32 columns: [wgT 0:128 | woT blocks 128:640 | bo 640:642 |
    #   wiT 642:646 | ba 646 | convb 647 | bi(rows 0:2) 648 | bg(row 0)
    #   649:777 | csg_all 777:905 | csg_own 905:921 | mean_xs0 921]
    # packbf columns: [waT 0:128 | convm 128:1536 | xs0T 1536:1536+na |
    #   xs0ownT +local_a | img0 +wpad | wmask +wpad]
    PF = 922
    PBF = 1536 + na + local_a + 2 * wpad
    ob_xs0own = 1536 + na
    ob_img0 = ob_xs0own + local_a
    ob_wmask = ob_img0 + wpad
    local_m = local_a + 1   # +1: global colsum rides as an extra SpMM column
    t_bmat = nc.dram_tensor("bmat", [na, local_m], f8, kind="ExternalInput").ap()
    t_packbf = nc.dram_tensor("packbf", [128, PBF], bf16, kind="ExternalInput").ap()
    t_packf32 = nc.dram_tensor("packf32", [128, PF], f32, kind="ExternalInput").ap()
    t_out = nc.dram_tensor("out", [2, 1], f32, kind="ExternalOutput").ap()

    rg = [list(range(ncores))]
    bmat_r = t_bmat.rearrange("(t p) m -> t p m", p=128)
    bmat_p = t_bmat.rearrange("(t two p) m -> t p two m", two=2, p=128)

    with tile.TileContext(nc) as tc:
        with (
            tc.tile_pool(name="persist", bufs=1) as persist,
            tc.tile_pool(name="hsp", bufs=4) as hsp,
            tc.tile_pool(name="slabp", bufs=slab_bufs) as slabp,
            tc.tile_pool(name="xsp", bufs=2) as xsp,
            tc.tile_pool(name="convp", bufs=3) as convp,
            tc.tile_pool(name="hsfp", bufs=1) as hsfp,
            tc.tile_pool(name="agbp", bufs=2) as agbp,
            tc.tile_pool(name="stp", bufs=3) as stp,
            tc.tile_pool(name="smallp", bufs=12) as smallp,
            tc.tile_pool(name="dotp", bufs=3) as dotp,
            tc.tile_pool(name="catp", bufs=3) as catp,
            tc.tile_pool(name="dram", bufs=1, space="DRAM") as dram,
            tc.tile_pool(name="ps_spmm", bufs=1, space="PSUM") as ps_spmm,
            tc.tile_pool(name="ps_misc", bufs=2, space="PSUM") as ps_misc,
            tc.tile_pool(name="ps_conv", bufs=1, space="PSUM") as ps_conv,
            tc.tile_pool(name="ps_dot", bufs=1, space="PSUM") as ps_dot,
        ):
            # ---- persistent small weights ----
            wgT = persist.tile([DIM, DIM], f32, tag="wgT")
            wgT_bf = persist.tile([DIM, DIM], bf16, tag="wgT_bf")
            bg_row = persist.tile([1, DIM], f32, tag="bg_row")
            waT = persist.tile([DIM, DIM], bf16, tag="waT")
            ba_col = persist.tile([DIM, 1], f32, tag="ba_col")
            convm = persist.tile([DIM, KK * DIM], bf16, tag="convm")
            convb_col = persist.tile([DIM, 1], f32, tag="convb_col")
            woT_sb = persist.tile([DIM, 4 * DIM], f32, tag="woT_sb")
            bo_sb = persist.tile([DIM, 2], f32, tag="bo_sb")
            wiT_sb = persist.tile([DIM, 4], f32, tag="wiT_sb")
            bi_sb = persist.tile([2, 1], f32, tag="bi_sb")
            ones_row = persist.tile([1, DIM], bf16, tag="ones_row")
            ones_f32 = persist.tile([1, DIM], f32, tag="ones_f32")
            bg_bf = persist.tile([1, DIM], bf16, tag="bg_bf")
            xs0_full = persist.tile([128, na], bf16, tag="xs0_full")
            xs0_own = persist.tile([128, local_a], bf16, tag="xs0_own")
            img0 = persist.tile([128, wpad], bf16, tag="img0")
            wmask_sb = persist.tile([128, wpad], bf16, tag="wmask_sb")
            csgo_f32 = persist.tile([DIM, och], f32, tag="csgo_f32")
            csgo_bf = persist.tile([DIM, och], bf16, tag="csgo_bf")
            mean_xs0 = persist.tile([DIM, 1], f32, tag="mean_xs0")
            ba_hsp = persist.tile([DIM, 1], f32, tag="ba_hsp")
            bcache = (persist.tile([128, cache_pairs * 2 * local_m], f8,
                                   tag="bcache", name="bcache") if cache_pairs else None)

            nc.sync.dma_start(wgT[:], t_packf32[:, 0:128])
            nc.vector.tensor_copy(wgT_bf[:], wgT[:])
            nc.sync.dma_start(bg_row[:], t_packf32[0:1, 649:777])
            nc.sync.dma_start(waT[:], t_packbf[:, 0:128])
            nc.sync.dma_start(ba_col[:], t_packf32[:, 646:647])
            nc.sync.dma_start(convm[:], t_packbf[:, 128:1536])
            nc.sync.dma_start(convb_col[:], t_packf32[:, 647:648])
            nc.sync.dma_start(woT_sb[:], t_packf32[:, 128:640])
            nc.sync.dma_start(bo_sb[:], t_packf32[:, 640:642])
            nc.sync.dma_start(wiT_sb[:], t_packf32[:, 642:646])
            nc.sync.dma_start(bi_sb[:], t_packf32[0:2, 648:649])
            nc.sync.dma_start(csgo_f32[:], t_packf32[:, 905:921])
            nc.vector.tensor_copy(csgo_bf[:], csgo_f32[:])
            nc.sync.dma_start(mean_xs0[:], t_packf32[:, 921:922])
            nc.vector.tensor_scalar_mul(ba_hsp[:], ba_col[:], HSP_SCALE)
            nc.gpsimd.memset(ones_f32[:], 1.0)
            nc.vector.tensor_copy(ones_row[:], ones_f32[:])
            nc.vector.tensor_copy(bg_bf[:], bg_row[:])
            nc.sync.dma_start(xs0_full[:], t_packbf[:, 1536:1536 + na])
            nc.sync.dma_start(xs0_own[:],
                              t_packbf[:, ob_xs0own:ob_xs0own + local_a])
            nc.sync.dma_start(img0[:], t_packbf[:, ob_img0:ob_img0 + wpad])
            nc.sync.dma_start(wmask_sb[:], t_packbf[:, ob_wmask:ob_wmask + wpad])

            def spmm_layer(layer, rep, hs_src, dot_ps=None):
                """SpMM over own rows via fp8 DoubleRow; psum [dims, local_a].
                hs_src(kj) -> lhsT AP [128, 2, DIM] f8 for chunk pair kj.
                dot_ps, if given, accumulates cs.hs into [128,1] via the
                slab's extra cs column — same lhsT, no PE weight switch."""
                psums = [ps_spmm.tile([128, 512], f32, tag=f"spmm{mb}",
                                      name=f"spmm_r{rep}_l{layer}_{mb}")
                         for mb in range(n_mb)]
                kc0 = ach // 2 - cache_pairs
                for kj in range(ach // 2):
                    if cache_pairs and kj >= kc0:
                        slab = bcache[:, (kj - kc0) * 2 * local_m:
                                      (kj - kc0 + 1) * 2 * local_m]
                        if layer == 0:
                            eng = nc.sync if kj % 2 == 0 else nc.scalar
                            if dma_mode == "pair1":
                                eng.dma_start(
                                    slab.rearrange("p (two m) -> p two m", two=2),
                                    bmat_p[kj])
                            else:
                                eng.dma_start(slab[:, :local_m], bmat_r[2 * kj])
                                eng.dma_start(slab[:, local_m:], bmat_r[2 * kj + 1])
                        slab3 = slab.rearrange("p (two m) -> p two m", two=2)
                    else:
                        slab = slabp.tile([128, 2 * local_m], f8, tag="slab")
                        if dma_mode == "pair1":
                            eng = nc.sync if kj % 2 == 0 else nc.scalar
                            eng.dma_start(
                                slab[:].rearrange("p (two m) -> p two m", two=2),
                                bmat_p[kj])
                        elif dma_mode == "sp":
                            nc.sync.dma_start(slab[:, :local_m], bmat_r[2 * kj])
                            nc.sync.dma_start(slab[:, local_m:], bmat_r[2 * kj + 1])
                        elif dma_mode == "alt":
                            eng = nc.sync if kj % 2 == 0 else nc.scalar
                            eng.dma_start(slab[:, :local_m], bmat_r[2 * kj])
                            eng.dma_start(slab[:, local_m:], bmat_r[2 * kj + 1])
                        else:  # "half": one half per engine every pair
                            nc.sync.dma_start(slab[:, :local_m], bmat_r[2 * kj])
                            nc.scalar.dma_start(slab[:, local_m:], bmat_r[2 * kj + 1])
                        slab3 = slab[:].rearrange("p (two m) -> p two m", two=2)
                    lhsT = hs_src(kj)
                    for mb in range(n_mb):
                        nc.tensor.matmul(
                            psums[mb][:], lhsT, slab3[:, :, mb * 512:(mb + 1) * 512],
                            start=(kj == 0), stop=(kj == ach // 2 - 1),
                            perf_mode=DR)
                    if dot_ps is not None:
                        nc.tensor.matmul(
                            dot_ps[:, :1], lhsT,
                            slab3[:, :, local_a:local_a + 1],
                            start=(kj == 0), stop=(kj == ach // 2 - 1),
                            perf_mode=DR)
                return psums

            def own_hs(xs_c, rep, layer):
                """hs (fp8, scaled) for own atoms from xs_c [128(dim), local_a]."""
                hso = hsp.tile([128, och * 128], f8, tag="hso",
                               name=f"hso_r{rep}_l{layer}")
                for t in range(och):
                    hp = ps_misc.tile([128, 256], f32, tag="m256")
                    nc.tensor.matmul(hp[:, :128], ones_f32[:], bg_row[:],
                                     start=True, stop=False)
                    nc.tensor.matmul(hp[:, :128], xs_c[:, t * 128:(t + 1) * 128],
                                     wgT[:], start=False, stop=True)
                    nc.scalar.activation(hso[:, t * 128:(t + 1) * 128], hp[:, :128],
                                         Relu, scale=HS_SCALES[layer])
                return hso

            def allgather_hs(hso, rep, layer):
                """own hs f8 [128, 2048] -> hs_full f8 [128, ach*128].
                All DMAs on gpsimd queue (keeps SP/ACT slab prefetch clear)."""
                agi = dram.tile([128, och * 128], f8, tag=f"agi_r{rep}_l{layer}",
                                name=f"agi_r{rep}_l{layer}")
                ago = dram.tile([128 * ncores, och * 128], f8,
                                tag=f"ago_r{rep}_l{layer}",
                                name=f"ago_r{rep}_l{layer}", addr_space="Shared")
                nc.gpsimd.dma_start(agi[:], hso[:])
                nc.gpsimd.collective_compute(
                    "AllGather", mybir.AluOpType.bypass,
                    ins=[agi[:].opt()], outs=[ago[:].opt()], replica_groups=rg)
                hs_full = hsfp.tile([128, ach * 128], f8, tag="hsfull",
                                    name=f"hsfull_r{rep}_l{layer}")
                for c in range(ncores):
                    nc.gpsimd.dma_start(
                        hs_full[:, c * och * 128:(c + 1) * och * 128],
                        ago[c * 128:(c + 1) * 128, :])
                return hs_full

            def xs_update(psums, base_bf, rep, layer):
                """xs_new f32 = base + psum/HS_SCALES[layer]."""
                xs_c = xsp.tile([128, local_a], f32, tag="xs_c",
                                name=f"xs_r{rep}_l{layer}")
                for mb in range(n_mb):
                    nc.vector.scalar_tensor_tensor(
                        xs_c[:, mb * 512:(mb + 1) * 512], psums[mb][:],
                        1.0 / HS_SCALES[layer], base_bf[:, mb * 512:(mb + 1) * 512],
                        op0=Mult, op1=Add)
                return xs_c

            # ---------------- protein conv pieces ----------------
            def conv_layer(src, l, rep):
                lo = WINDOW * (l + 1)
                hi = lwin - WINDOW * (l + 1)
                dst = convp.tile([128, wpad], bf16, tag="convb",
                                 name=f"conv_r{rep}_l{l}")
                o = lo
                while o < hi:
                    ms = min(512, hi - o)
                    pc = ps_conv.tile([128, 512], f32, tag="pconv")
                    for a in range(KK):
                        nc.tensor.matmul(
                            pc[:, :ms], convm[:, a * DIM:(a + 1) * DIM],
                            src[:, o + a - WINDOW:o + a - WINDOW + ms],
                            start=(a == 0), stop=(a == KK - 1))
                    nc.scalar.activation(dst[:, o:o + ms], pc[:, :ms], Relu,
                                         bias=convb_col[:])
                    o += ms
                if l < LAYER_CNN - 1:
                    nc.vector.tensor_tensor(dst[:, lo:hi], dst[:, lo:hi],
                                            wmask_sb[:, lo:hi], op=Mult)
                return dst

            def protein_hsp(xsp_img, dst):
                """hs_p = relu(Wa xs_p + ba) for own words -> dst[:, :local_w]."""
                o = 0
                while o < local_w:
                    ms = min(512, local_w - o)
                    pc = ps_conv.tile([128, 512], f32, tag="pconv")
                    nc.tensor.matmul(pc[:, :ms], waT[:],
                                     xsp_img[:, HALO + o:HALO + o + ms],
                                     start=True, stop=True)
                    nc.scalar.activation(dst[:, o:o + ms], pc[:, :ms], Relu,
                                         scale=HSP_SCALE, bias=ba_hsp[:])
                    o += ms

            def tail(rep, dot0_sb, dot1_sb, dots8, hs_p_full):
                # dot2 = sum of gathered f32 partials
                d2s = smallp.tile([128, 1], f32, tag="small")
                nc.vector.reduce_sum(d2s[:], dots8[:], axis=AxX)
                t0 = smallp.tile([128, 1], f32, tag="small")
                nc.vector.tensor_tensor(t0[:], dot0_sb[:], dot1_sb[:], op=Add)
                t1 = smallp.tile([128, 1], f32, tag="small")
                nc.vector.tensor_tensor(t1[:], d2s[:], t0[:], op=Add)
                comp = smallp.tile([128, 1], f32, tag="small")
                nc.vector.scalar_tensor_tensor(comp[:], t1[:], 1.0 / na,
                                               mean_xs0[:], op0=Mult, op1=Add)
                # h = relu(Wa compound + ba)
                comp_bf = smallp.tile([128, 1], bf16, tag="smallbf")
                nc.vector.tensor_copy(comp_bf[:], comp[:])
                ph = ps_conv.tile([128, 512], f32, tag="pconv")
                nc.tensor.matmul(ph[:, :1], waT[:], comp_bf[:], start=True,
                                 stop=True)
                h_bf = smallp.tile([128, 1], bf16, tag="smallbf")
                nc.scalar.activation(h_bf[:], ph[:, :1], Relu, bias=ba_col[:])
                # exact attention over ALL nw words (hs_p_full gathered, fp8
                # scaled by HSP_SCALE; tanh arg rescaled inside activation)
                prev = None
                o = 0
                while o < nw:
                    ms = min(512, nw - o)
                    pw = ps_misc.tile([128, 512], f32, tag="m256")
                    nc.tensor.matmul(pw[:1, :ms], h_bf[:],
                                     hs_p_full[:, o:o + ms], start=True, stop=True)
                    w_bf = stp.tile([1, 512], bf16, tag="wrow")
                    nc.scalar.activation(w_bf[:, :ms], pw[:1, :ms], Tanh,
                                         scale=1.0 / HSP_SCALE)
                    pb = ps_conv.tile([128, 512], f32, tag="pconv")
                    nc.tensor.matmul(pb[:, :ms], ones_row[:], w_bf[:, :ms],
                                     start=True, stop=True)
                    scr = stp.tile([128, 512], f32, tag="ysscr")
                    nc.vector.tensor_tensor(scr[:, :ms], pb[:, :ms],
                                            hs_p_full[:, o:o + ms], op=Mult)
                    acc = smallp.tile([128, 1], f32, tag="small")
                    nc.vector.reduce_sum(acc[:], scr[:, :ms], axis=AxX)
                    if prev is not None:
                        nc.vector.tensor_tensor(acc[:], acc[:], prev[:], op=Add)
                    prev = acc
                    o += ms
                # output MLP on cat = [compound, protein]
                cat = catp.tile([128, 2], f32, tag="cat")
                nc.vector.tensor_copy(cat[:, 0:1], comp[:])
                nc.vector.tensor_scalar_mul(cat[:, 1:2], prev[:],
                                            1.0 / (nw * HSP_SCALE))
                for l in range(LAYER_OUT):
                    ncat = catp.tile([128, 2], f32, tag="cat")
                    for i in range(2):
                        pm = ps_misc.tile([128, 256], f32, tag="m256")
                        for j in range(2):
                            nc.tensor.matmul(
                                pm[:, :1],
                                woT_sb[:, (j * 2 + i) * DIM:(j * 2 + i + 1) * DIM],
                                cat[:, j:j + 1], start=(j == 0), stop=(j == 1))
                        nc.scalar.activation(ncat[:, i:i + 1], pm[:, :1], Relu,
                                             bias=bo_sb[:, i:i + 1])
                    cat = ncat
                pf = ps_misc.tile([128, 256], f32, tag="m256")
                for j in range(2):
                    nc.tensor.matmul(pf[:2, :1], wiT_sb[:, 2 * j:2 * j + 2],
                                     cat[:, j:j + 1], start=(j == 0), stop=(j == 1))
                res = smallp.tile([2, 1], f32, tag="res")
                nc.scalar.activation(res[:], pf[:2, :1], Ident, bias=bi_sb[:])
                nc.sync.dma_start(t_out[:], res[:])

            def finish_early():
                res2 = smallp.tile([2, 1], f32, tag="res")
                nc.vector.tensor_copy(res2[:], bi_sb[:])
                nc.sync.dma_start(t_out[:], res2[:])

            # ================= main body (reps > 1 for timing) =================
            for rep in range(reps):
                if stage < 2:
                    break
                # ---- GNN layer 0 SpMM; dot0 = cs.hs0 accumulated in-loop ----
                pd0 = ps_dot.tile([128, 128], f32, tag="pdot",
                                  name=f"pd0_r{rep}")

                def hs0_pair(kj):
                    hp = ps_misc.tile([128, 256], f32, tag="m256")
                    hch = hsp.tile([128, 256], f8, tag="hs0c")
                    for i in range(2):
                        ki = 2 * kj + i
                        nc.tensor.matmul(hp[:, i * 128:(i + 1) * 128],
                                         ones_row[:], bg_bf[:], start=True,
                                         stop=False)
                        nc.tensor.matmul(hp[:, i * 128:(i + 1) * 128],
                                         xs0_full[:, ki * 128:(ki + 1) * 128],
                                         wgT_bf[:], start=False, stop=True)
                    nc.scalar.activation(hch[:], hp[:], Relu, scale=HS_SCALES[0])
                    return hch[:].rearrange("p (two d) -> p two d", two=2)

                psums0 = spmm_layer(0, rep, hs0_pair, dot_ps=pd0)
                xs1_c = xs_update(psums0, xs0_own, rep, 0)
                dot0_sb = dotp.tile([128, 1], f32, tag="dotcol")
                nc.scalar.activation(dot0_sb[:], pd0[:, :1], Ident,
                                     scale=1.0 / HS_SCALES[0])
                if stage < 3:
                    continue
                # ---- own hs1; conv layer 1 fills the AllGather gap ----
                hso1 = own_hs(xs1_c, rep, 1)
                imgb = conv_layer(img0, 0, rep)
                if stage < 4:
                    continue
                hs1_full = allgather_hs(hso1, rep, 1)
                # protein side is GNN-independent: finish it in the AG gap
                agbt = agbp.tile([128, local_w], f8, tag="agbt",
                                 name=f"agbt_r{rep}")
                imgc = conv_layer(imgb, 1, rep)
                imgd = conv_layer(imgc, 2, rep)
                protein_hsp(imgd, agbt)
                # AllGather hs_p NOW — it hides under the layer-1 SpMM
                agbi = dram.tile([128, local_w], f8, tag=f"agbi_r{rep}",
                                 name=f"agbi_r{rep}")
                agbo = dram.tile([128 * ncores, local_w], f8,
                                 tag=f"agbo_r{rep}", name=f"agbo_r{rep}",
                                 addr_space="Shared")
                nc.gpsimd.dma_start(agbi[:], agbt[:])
                nc.gpsimd.collective_compute(
                    "AllGather", mybir.AluOpType.bypass,
                    ins=[agbi[:].opt()], outs=[agbo[:].opt()], replica_groups=rg)
                hs_p_full = hsfp.tile([128, nw], f8, tag="hspfull",
                                      name=f"hspf_r{rep}")
                for c in range(ncores):
                    eng = (nc.gpsimd, nc.sync, nc.scalar)[c % 3]
                    eng.dma_start(
                        hs_p_full[:, c * local_w:(c + 1) * local_w],
                        agbo[c * 128:(c + 1) * 128, :])
                # ---- GNN layer 1 SpMM; dot1 = cs.hs1 accumulated in-loop ----
                pd1 = ps_dot.tile([128, 128], f32, tag="pdot",
                                  name=f"pd1_r{rep}")
                hs1_r = hs1_full[:].rearrange("p (t d) -> p t d", d=128)
                psums1 = spmm_layer(1, rep,
                                    lambda kj: hs1_r[:, 2 * kj:2 * kj + 2, :],
                                    dot_ps=pd1)
                xs2_c = xs_update(psums1, xs1_c, rep, 1)
                dot1_sb = dotp.tile([128, 1], f32, tag="dotcol")
                nc.scalar.activation(dot1_sb[:], pd1[:, :1], Ident,
                                     scale=1.0 / HS_SCALES[1])
                if stage < 5:
                    continue
                # ---- layer 2: hs2 own, dot2 partial, joint AllGather ----
                hso2 = own_hs(xs2_c, rep, 2)
                pd2 = ps_dot.tile([128, 128], f32, tag="pdot",
                                  name=f"pd2_r{rep}")
                for t in range(och):
                    nc.tensor.matmul(pd2[:, :1],
                                     hso2[:, t * 128:(t + 1) * 128],
                                     csgo_bf[:, t:t + 1],
                                     start=(t == 0), stop=(t == och - 1))
                d2 = smallp.tile([128, 1], f32, tag="small")
                nc.scalar.activation(d2[:], pd2[:, :1], Ident,
                                     scale=1.0 / HS_SCALES[2])
                # tiny exact f32 AllGather of the dot2 partials
                agci = dram.tile([128, 1], f32, tag=f"agci_r{rep}",
                                 name=f"agci_r{rep}")
                agco = dram.tile([128 * ncores, 1], f32, tag=f"agco_r{rep}",
                                 name=f"agco_r{rep}", addr_space="Shared")
                nc.gpsimd.dma_start(agci[:], d2[:])
                nc.gpsimd.collective_compute(
                    "AllGather", mybir.AluOpType.bypass,
                    ins=[agci[:].opt()], outs=[agco[:].opt()], replica_groups=rg)
                dots8 = smallp.tile([128, ncores], f32, tag="dots8")
                for c in range(ncores):
                    nc.gpsimd.dma_start(dots8[:, c:c + 1],
                                        agco[c * 128:(c + 1) * 128, :])
                if stage < 7:
                    continue
                tail(rep, dot0_sb, dot1_sb, dots8, hs_p_full)

            if stage < 7:
                finish_early()

    nc.compile()
    return nc


def prep_in_maps(inputs, na, nw, nfp, nword, ncores):
    """Host-side sharding/layout prep (gathers + casts + transposes)."""
    local_a = na // ncores
    local_w = nw // ncores
    lwin = local_w + 2 * HALO
    wch = _ceil_div(lwin, 128)
    wpad = wch * 128

    fingerprints = np.asarray(inputs["fingerprints"]).astype(np.int64)
    adjacency = np.asarray(inputs["adjacency"], dtype=np.float32)
    words = np.asarray(inputs["words"]).astype(np.int64)
    embed_fp = np.asarray(inputs["embed_fp"], dtype=np.float32)
    embed_word = np.asarray(inputs["embed_word"], dtype=np.float32)
    Wg = np.asarray(inputs["Wg"], dtype=np.float32)
    bg = np.asarray(inputs["bg"], dtype=np.float32)
    conv_k = np.asarray(inputs["conv_k"], dtype=np.float32)
    conv_b = np.asarray(inputs["conv_b"], dtype=np.float32)
    Wa = np.asarray(inputs["Wa"], dtype=np.float32)
    ba = np.asarray(inputs["ba"], dtype=np.float32)
    Wo = np.asarray(inputs["Wo"], dtype=np.float32)
    bo = np.asarray(inputs["bo"], dtype=np.float32)
    Wi = np.asarray(inputs["Wi"], dtype=np.float32)
    bi = np.asarray(inputs["bi"], dtype=np.float32)

    # adjacency -> fp8 bit pattern (0.0 -> 0x00, nonzero -> 0x38 = 1.0 in e4m3)
    nz = adjacency != 0
    a8 = nz.astype(np.uint8) * np.uint8(0x38)
    colsum_g = nz.sum(axis=0).astype(np.float32)               # [na]

    # xs0 gather, dim-major, bf16 (replicated)
    xs0 = embed_fp[fingerprints]                       # [na, DIM] f32
    xs0T = np.ascontiguousarray(xs0.T).astype(BF16)    # [128, na]

    K2 = conv_k[0, 0]
    M = np.zeros((DIM, KK * DIM), np.float32)
    for a in range(KK):
        Ma = np.zeros((DIM, DIM), np.float32)
        for b_ in range(KK):
            Ma += K2[a, b_] * np.eye(DIM, k=5 - b_, dtype=np.float32)
        M[:, a * DIM:(a + 1) * DIM] = Ma

    # packf32 [128, 922]: layouts the kernel consumes directly
    PF = 922
    pf = np.zeros((DIM, PF), np.float32)
    pf[:, 0:128] = Wg.T
    WoT = Wo.T.astype(np.float32)
    WiT = Wi.T.astype(np.float32)
    for j in range(2):
        for i in range(2):
            pf[:, 128 + (j * 2 + i) * DIM:128 + (j * 2 + i + 1) * DIM] = \
                WoT[j * DIM:(j + 1) * DIM, i * DIM:(i + 1) * DIM]
        pf[:, 640 + j] = bo[j * DIM:(j + 1) * DIM]
        pf[:, 642 + 2 * j:644 + 2 * j] = WiT[j * DIM:(j + 1) * DIM, :]
    pf[:, 646] = ba
    pf[:, 647] = conv_b[0]
    pf[0:2, 648] = bi
    pf[0, 649:777] = bg
    pf[:, 777:905] = colsum_g.reshape(na // 128, 128).T
    pf[:, 921] = xs0.mean(axis=0)

    ws_full = embed_word[words]                        # [nw, DIM] f32

    PBF = 1536 + na + local_a + 2 * wpad
    ob_xs0own = 1536 + na
    ob_img0 = ob_xs0own + local_a
    ob_wmask = ob_img0 + wpad

    in_maps = []
    for c in range(ncores):
        sl = slice(c * local_a, (c + 1) * local_a)
        bm = np.empty((na, local_a + 1), np.uint8)
        bm[:, :local_a] = a8[sl, :].T
        bm[:, local_a] = colsum_g.astype(F8).view(np.uint8)
        bmat = np.ascontiguousarray(bm).view(F8)
        w0 = c * local_w - HALO
        pos = np.arange(wpad)
        gidx = w0 + pos
        valid = (gidx >= 0) & (gidx < nw) & (pos < lwin)
        win = np.where(valid[:, None], ws_full[np.clip(gidx, 0, nw - 1)], 0.0)
        pb = np.zeros((DIM, PBF), BF16)
        pb[:, 0:128] = Wa.T.astype(BF16)
        pb[:, 128:1536] = M.astype(BF16)
        pb[:, 1536:1536 + na] = xs0T
        pb[:, ob_xs0own:ob_xs0own + local_a] = xs0T[:, sl]
        pb[:, ob_img0:ob_img0 + wpad] = win.T.astype(BF16)
        pb[:, ob_wmask:ob_wmask + wpad] = \
            ((gidx >= 0) & (gidx < nw)).astype(BF16)[None, :]
        pfc = pf.copy()
        pfc[:, 905:921] = colsum_g[sl].reshape(local_a // 128, 128).T
        in_maps.append(dict(bmat=bmat, packbf=pb, packf32=pfc))
    return in_maps


_CACHE = {}


def _get_kernel(cfg_key):
    if cfg_key not in _CACHE:
        na, nw, nfp, nword, ncores = cfg_key
        _CACHE[cfg_key] = build_kernel(na, nw, nfp, nword, ncores)
    return _CACHE[cfg_key]


def kernel(**inputs) -> np.ndarray:
    from concourse import bass_utils
    cfg = FULL
    key = (cfg["na"], cfg["nw"], cfg["nfp"], cfg["nword"], cfg["ncores"])
    nc = _get_kernel(key)
    in_maps = prep_in_maps(inputs, *key)
    res = bass_utils.run_bass_kernel_spmd(
        nc, in_maps, core_ids=list(range(cfg["ncores"])), trace=False)
    out = np.asarray(res.results[0]["out"], np.float32).reshape(1, 2)
    return out
